# revision 22
# baseline (speedup 1.0000x reference)
"""TransformerConv GNN (3 layers) on 8 Trainium2 NeuronCores.

Sharding: nodes split 3750/core (padded to 3840 = 30 tiles of 128).
Edges assigned to the core owning their dst node, grouped by 128-node
dst windows.

Per layer:
  node phase: ln1 (folded into weights host-side); fused q|k|v|skip
    projection as ONE bf16 matmul per tile ([D, 4D] moving operand);
    transposes via the DMA xbar (dma_start_transpose), not the PE;
    interleaved k|v table written to HBM (bf16).
  kv exchange: AllGather of the per-core kv shard (bf16).
  edge phase: dma_gather of kv[src] only. Per-edge q comes from a PE
    matmul with the transposed dst one-hot as stationary (dst is
    window-local). The scatter one-hot is generated on-chip (DVE
    is_equal vs an iota row). Edge-attr projection e goes PE->PSUM;
    k+e / v+e computed by one DVE add into SBUF bf16; alpha/exp/message
    on DVE+ACT; segment softmax accumulated per dst window via one-hot
    matmuls into PSUM.
  FFN phase + next layer's node phase are interleaved into the edge
    phase in groups of 10 dst windows, so they hide under the
    Q7-gather-bound edge phase; activations batched per group to keep
    ACT function-table reloads rare.
Output head node-local (also per group); host reassembles shards.
"""
import contextlib
import math
import os
import numpy as np

import concourse.bass as bass
import concourse.bacc as bacc
import concourse.tile as tile
from concourse import mybir, library_config
from concourse.bass_utils import run_bass_kernel_spmd

# problem dims
N, E, F, D, H, C, ED, L = 30000, 300000, 64, 128, 8, 16, 16, 3
NCORES = 8
NL = N // NCORES          # 3750 real nodes per core
NT = 30                   # node tiles per core
NLP = NT * 128            # 3840 padded nodes per core
KVROWS = NCORES * NLP     # kv table rows (global)
P = 128
G = 8                     # edge tiles per gather batch (dma_gather <=1024 idxs)
B = 4                     # edge tiles per DVE op group
GW = 10                   # dst windows per interleaved node/FFN group

fp32 = mybir.dt.float32
bf16 = mybir.dt.bfloat16
i16 = mybir.dt.int16

AF = mybir.ActivationFunctionType
OP = mybir.AluOpType
AX = mybir.AxisListType

LAST_RESULT = None


def _bcast3(ap, reps):
    """[P, k] AP -> [P, k, reps] with 0-stride last dim."""
    return bass.AP(tensor=ap.tensor, offset=ap.offset,
                   ap=[ap.ap[0], ap.ap[1], [0, reps]])


def _bcast4(ap, reps):
    """[P, b, k] AP -> [P, b, k, reps] with 0-stride last dim."""
    return bass.AP(tensor=ap.tensor, offset=ap.offset,
                   ap=[ap.ap[0], ap.ap[1], ap.ap[2], [0, reps]])


def _bcast_mid(ap, reps):
    """[P, k] AP -> [P, reps, k] with 0-stride middle dim."""
    return bass.AP(tensor=ap.tensor, offset=ap.offset,
                   ap=[ap.ap[0], [0, reps], ap.ap[1]])


def build(tiles_per_window, skip_bias):
    """Build the Bass program. tiles_per_window: NT ints, same per core."""
    assert skip_bias, "non-zero bias path not implemented"
    tot_tiles = sum(tiles_per_window)
    tot_e = tot_tiles * 128
    nbatch = math.ceil(tot_tiles / G)

    tile_win, win_first, win_last = [], [], []
    for w, tw in enumerate(tiles_per_window):
        for i in range(tw):
            tile_win.append(w)
            win_first.append(i == 0)
            win_last.append(i == tw - 1)

    nc = bacc.Bacc("TRN2", target_bir_lowering=False, debug=False,
                   num_devices=NCORES)

    # ---------------- DRAM tensors ----------------
    x_in = nc.dram_tensor("x_shard", [NLP, F], fp32, kind="ExternalInput").ap()
    idx_src_d = nc.dram_tensor("idx_src", [P, tot_e // 16], i16,
                               kind="ExternalInput").ap()
    ohT_d = nc.dram_tensor("ohT", [P, tot_e], bf16, kind="ExternalInput").ap()
    dcol_d = nc.dram_tensor("dst_col", [P, tot_tiles], bf16,
                            kind="ExternalInput").ap()
    rix_d = nc.dram_tensor("rowidx", [P, P], bf16, kind="ExternalInput").ap()
    ea_d = nc.dram_tensor("ea_t", [ED, tot_e], bf16, kind="ExternalInput").ap()
    w4_d = nc.dram_tensor("w4T", [L, D, 4 * D], bf16,
                          kind="ExternalInput").ap()
    w1_d = nc.dram_tensor("w1T", [L, D, D], bf16, kind="ExternalInput").ap()
    w2_d = nc.dram_tensor("w2T", [L, D, D], bf16, kind="ExternalInput").ap()
    ewd_d = nc.dram_tensor("ewdT", [L, ED, 2 * D], bf16,
                           kind="ExternalInput").ap()
    w0_d = nc.dram_tensor("w0T", [F, D], fp32, kind="ExternalInput").ap()
    id_d = nc.dram_tensor("ident", [P, P], fp32, kind="ExternalInput").ap()
    wl_d = nc.dram_tensor("wlT", [D, 4], bf16, kind="ExternalInput").ap()
    out_d = nc.dram_tensor("out", [NLP, 4], fp32, kind="ExternalOutput").ap()

    kv_bounce = nc.dram_tensor("kv_bounce", [NLP, 2 * D], bf16).ap()
    kv_full = nc.dram_tensor("kv_full", [KVROWS, 2 * D], bf16,
                             addr_space="Shared").ap()

    eps = 1e-5

    # group boundaries: window index at which each group completes
    grp_end_w = [GW - 1 + GW * i for i in range(NT // GW)]  # [9, 19, 29]

    with tile.TileContext(nc) as tc:
        nc.gpsimd.load_library(library_config.mlp)
        with contextlib.ExitStack() as ctx:
            const = ctx.enter_context(tc.tile_pool(name="const", bufs=1))
            nodes = ctx.enter_context(tc.tile_pool(name="nodes", bufs=1))
            ntmp = ctx.enter_context(tc.tile_pool(name="ntmp", bufs=3))
            nsm = ctx.enter_context(tc.tile_pool(name="nsm", bufs=4))
            gtmp = ctx.enter_context(tc.tile_pool(name="gtmp", bufs=2))
            gbuf = ctx.enter_context(tc.tile_pool(name="gbuf", bufs=3))
            ebuf = ctx.enter_context(tc.tile_pool(name="ebuf", bufs=3))
            # PSUM pools (live across the whole kernel)
            pps = ctx.enter_context(
                tc.tile_pool(name="pps", bufs=1, space="PSUM"))

            # constants
            id32 = const.tile([P, P], fp32, tag="id32")
            nc.sync.dma_start(out=id32[:], in_=id_d[:, :])
            id16 = const.tile([P, P], bf16, tag="id16")
            nc.vector.tensor_copy(out=id16[:], in_=id32[:])
            eps_t = const.tile([P, 1], fp32, tag="eps")
            nc.vector.memset(eps_t[:], eps)
            rix = const.tile([P, P], bf16, tag="rix")
            nc.sync.dma_start(out=rix[:], in_=rix_d[:, :])
            dcol = const.tile([P, tot_tiles], bf16, tag="dcol")
            nc.sync.dma_start(out=dcol[:], in_=dcol_d[:, :])
            idx_src = const.tile([P, tot_e // 16], i16, tag="isrc")
            nc.sync.dma_start(out=idx_src[:], in_=idx_src_d[:, :])
            w0 = const.tile([F, D], fp32, tag="w0")
            nc.sync.dma_start(out=w0[:], in_=w0_d[:, :])
            wl = const.tile([D, 4], bf16, tag="wl")
            nc.sync.dma_start(out=wl[:], in_=wl_d[:, :])
            w4 = const.tile([D, L, 4 * D], bf16, tag="w4")
            w1 = const.tile([D, L, D], bf16, tag="w1")
            w2 = const.tile([D, L, D], bf16, tag="w2")
            ewd = const.tile([ED, L, 2 * D], bf16, tag="ewd")
            for l in range(L):
                nc.sync.dma_start(out=w4[:, l, :], in_=w4_d[l])
                nc.sync.dma_start(out=w1[:, l, :], in_=w1_d[l])
                nc.sync.dma_start(out=w2[:, l, :], in_=w2_d[l])
                nc.sync.dma_start(out=ewd[:, l, :], in_=ewd_d[l])

            acc_t = pps.tile([P, 2, D + 8], fp32, space="PSUM", tag="acc",
                             bufs=1, name="acc_t")
            h_t = nodes.tile([P, NT, D], fp32, tag="h")
            skip_t = nodes.tile([P, NT, D], fp32, tag="skip")
            idn_t = nodes.tile([P, NT, D], fp32, tag="idn")
            hc_t = nodes.tile([P, NT, D], bf16, tag="hc")
            q_sb = nodes.tile([P, NT, D], bf16, tag="qsb")

            def transpose16(src_ap, name):
                tp = pps.tile([P, P], bf16, space="PSUM", tag="tp",
                              bufs=1, name="tp_" + name)
                nc.tensor.transpose(out=tp[:], in_=src_ap, identity=id16[:])
                dst = ntmp.tile([P, P], bf16, tag="trT", name=name)
                nc.scalar.copy(out=dst[:], in_=tp[:])
                return dst

            def ln_group(src_t, ts, out_b):
                """Batched layernorm of src_t[:, t, :] for t in ts into
                out_b[:, i, :] (bf16)."""
                n = len(ts)
                sts = gtmp.tile([P, GW, 6], fp32, tag="sts", name="sts")
                mvs = gtmp.tile([P, GW, 2], fp32, tag="mvs", name="mvs")
                for i, t in enumerate(ts):
                    nc.vector.bn_stats(out=sts[:, i, :], in_=src_t[:, t, :])
                    nc.vector.bn_aggr(out=mvs[:, i, :], in_=sts[:, i, :])
                sd = gtmp.tile([P, GW], fp32, tag="sd", name="sd")
                nc.scalar.activation(out=sd[:, :n], in_=mvs[:, :n, 1:2],
                                     func=AF.Ln, bias=eps_t[:], scale=1.0)
                rs = gtmp.tile([P, GW], fp32, tag="rs", name="rs")
                nc.scalar.activation(out=rs[:, :n], in_=sd[:, :n],
                                     func=AF.Exp, scale=-0.5)
                for i, t in enumerate(ts):
                    nc.vector.scalar_tensor_tensor(
                        out=out_b[:, i, :], in0=src_t[:, t, :],
                        scalar=mvs[:, i, 0:1],
                        in1=rs[:, i:i + 1].to_broadcast([P, D]),
                        op0=OP.subtract, op1=OP.mult)

            def node_group(l, ts):
                """ln1 + fused qkvs projection for tiles ts of layer l."""
                hnb = gtmp.tile([P, GW, D], bf16, tag="hnb", name="hnb")
                ln_group(h_t, ts, hnb)
                for i, t in enumerate(ts):
                    hnT = transpose16(hnb[:, i, :], "hnT")
                    p4 = pps.tile([P, 4 * D], fp32, space="PSUM", tag="p4",
                                  bufs=1, name="p4")
                    nc.tensor.matmul(out=p4[:], lhsT=hnT[:], rhs=w4[:, l, :],
                                     start=True, stop=True)
                    nc.scalar.copy(out=q_sb[:, t, :], in_=p4[:, :D])
                    kvb = ntmp.tile([P, 2 * D], bf16, tag="kvb", name="kvb")
                    nc.scalar.copy(out=kvb[:], in_=p4[:, D:3 * D])
                    nc.sync.dma_start(out=kv_bounce[t * P:(t + 1) * P, :],
                                      in_=kvb[:])
                    nc.vector.tensor_copy(out=skip_t[:, t, :],
                                          in_=p4[:, 3 * D:])

            def ffn_a_group(l, ts):
                """idn = gelu(hc @ w1) + h for tiles ts."""
                for t in ts:
                    hcT = transpose16(hc_t[:, t, :], "hcT")
                    t1p = pps.tile([P, D], fp32, space="PSUM", tag="mm",
                                   bufs=2, name="t1p")
                    nc.tensor.matmul(out=t1p[:], lhsT=hcT[:], rhs=w1[:, l, :],
                                     start=True, stop=True)
                    t1g = ntmp.tile([P, D], fp32, tag="t1g", name="t1g")
                    nc.scalar.activation(out=t1g[:], in_=t1p[:], func=AF.Gelu)
                    nc.vector.tensor_tensor(out=idn_t[:, t, :], in0=t1g[:],
                                            in1=h_t[:, t, :], op=OP.add)

            def ffn_b_group(l, ts):
                """h = elu(ln2(idn) @ w2) + idn for tiles ts."""
                t2b = gtmp.tile([P, GW, D], bf16, tag="hnb", name="t2b")
                ln_group(idn_t, ts, t2b)
                for i, t in enumerate(ts):
                    t2T = transpose16(t2b[:, i, :], "t2T")
                    t3p = pps.tile([P, D], fp32, space="PSUM", tag="mm",
                                   bufs=2, name="t3p")
                    nc.tensor.matmul(out=t3p[:], lhsT=t2T[:], rhs=w2[:, l, :],
                                     start=True, stop=True)
                    mn = nsm.tile([P, D], fp32, tag="mn", name="mn2")
                    nc.vector.tensor_scalar_min(mn[:], t3p[:], 0.0)
                    em = nsm.tile([P, D], fp32, tag="em", name="em2")
                    nc.scalar.activation(out=em[:], in_=mn[:], func=AF.Exp)
                    mx = nsm.tile([P, D], fp32, tag="mx", name="mx2")
                    nc.vector.tensor_scalar_max(mx[:], t3p[:], 0.0)
                    t4 = nsm.tile([P, D], fp32, tag="t4", name="t4")
                    nc.vector.scalar_tensor_tensor(
                        out=t4[:], in0=em[:], scalar=-1.0, in1=mx[:],
                        op0=OP.add, op1=OP.add)
                    nc.vector.tensor_tensor(out=h_t[:, t, :], in0=t4[:],
                                            in1=idn_t[:, t, :], op=OP.add)

            def head_group(ts):
                """Final layernorm + output projection for tiles ts."""
                hlb = gtmp.tile([P, GW, D], bf16, tag="hnb", name="hlb")
                ln_group(h_t, ts, hlb)
                for i, t in enumerate(ts):
                    hlT = transpose16(hlb[:, i, :], "hlT")
                    opm = pps.tile([P, D], fp32, space="PSUM", tag="mm",
                                   bufs=2, name="opm")
                    op_ = opm[:, 0:4]
                    nc.tensor.matmul(out=op_, lhsT=hlT[:], rhs=wl[:],
                                     start=True, stop=True)
                    ot = ntmp.tile([P, 4], fp32, tag="ot", name="ot")
                    nc.scalar.copy(out=ot[:], in_=op_)
                    nc.sync.dma_start(out=out_d[t * P:(t + 1) * P, :],
                                      in_=ot[:])

            # ---------------- input projection ----------------
            for t in range(NT):
                xt = ntmp.tile([P, F], fp32, tag="xt", name="xt")
                nc.sync.dma_start(out=xt[:], in_=x_in[t * P:(t + 1) * P, :])
                tp = pps.tile([P, P], fp32, space="PSUM", tag="mm",
                              bufs=2, name="tp")
                nc.tensor.transpose(out=tp[:F, :], in_=xt[:], identity=id32[:])
                xT = ntmp.tile([F, P], fp32, tag="xT", name="xT")
                nc.vector.tensor_copy(out=xT[:], in_=tp[:F, :])
                h0 = pps.tile([P, D], fp32, space="PSUM", tag="mm",
                              bufs=2, name="h0")
                nc.tensor.matmul(out=h0[:], lhsT=xT[:], rhs=w0[:],
                                 start=True, stop=True)
                mn = nsm.tile([P, D], fp32, tag="mn", name="mn")
                nc.vector.tensor_scalar_min(mn[:], h0[:], 0.0)
                em = nsm.tile([P, D], fp32, tag="em", name="em")
                nc.scalar.activation(out=em[:], in_=mn[:], func=AF.Exp)
                mx = nsm.tile([P, D], fp32, tag="mx", name="mx")
                nc.vector.tensor_scalar_max(mx[:], h0[:], 0.0)
                nc.vector.scalar_tensor_tensor(
                    out=h_t[:, t, :], in0=em[:], scalar=-1.0, in1=mx[:],
                    op0=OP.add, op1=OP.add)

            # layer 0 node phase (later layers interleave into edge phase)
            for g0 in range(0, NT, GW):
                node_group(0, list(range(g0, g0 + GW)))

            # ---------------- layers ----------------
            for l in range(L):
                # ---- kv exchange ----
                nc.gpsimd.collective_compute(
                    "AllGather", OP.bypass,
                    replica_groups=[list(range(NCORES))],
                    ins=[kv_bounce.opt()], outs=[kv_full.opt()])

                # ---- edge phase with interleaved FFN / next node phase ----
                pending = []
                for g in range(nbatch):
                    if pending:
                        pending.pop(0)()
                    t0 = g * G
                    gb = min(G, tot_tiles - t0)
                    ne = gb * 128
                    kvg = gbuf.tile([P, G, 2 * D], bf16, tag="kvg",
                                    name="kvg")
                    nc.gpsimd.dma_gather(
                        kvg[:, :gb, :], kv_full[:],
                        idx_src[:, t0 * 8:t0 * 8 + ne // 16],
                        ne, ne, 2 * D)
                    eat = gbuf.tile([ED, G * 128], bf16, tag="eat",
                                    name="eat")
                    nc.sync.dma_start(
                        out=eat[:, :ne],
                        in_=ea_d[:, t0 * 128:t0 * 128 + ne])
                    oh = gbuf.tile([P, G, P], bf16, tag="oh", name="oh")
                    nc.vector.tensor_tensor(
                        out=oh[:, :gb, :],
                        in0=_bcast_mid(rix[:], gb),
                        in1=_bcast3(dcol[:, t0:t0 + gb], P),
                        op=OP.is_equal)
                    ohTs = gbuf.tile([P, G, P], bf16, tag="ohTs",
                                     name="ohTs")
                    nc.sync.dma_start(
                        out=ohTs[:, :gb, :],
                        in_=ohT_d[:, t0 * 128:t0 * 128 + ne])

                    for bb in range(math.ceil(gb / B)):
                        nb = min(B, gb - bb * B)
                        qs = bb * B
                        ep = pps.tile([P, B, 2 * D], fp32, space="PSUM",
                                      tag="ep", bufs=1, name="ep")
                        qgp = pps.tile([P, B, D], fp32, space="PSUM",
                                       tag="qgp", bufs=1, name="qgp")
                        for u in range(nb):
                            te = qs + u
                            tid = t0 + te
                            nc.tensor.matmul(
                                out=ep[:, u, :],
                                lhsT=eat[:, te * 128:(te + 1) * 128],
                                rhs=ewd[:, l, :], start=True, stop=True,
                                skip_group_check=True)
                            nc.tensor.matmul(
                                out=qgp[:, u, :],
                                lhsT=ohTs[:, te, :],
                                rhs=q_sb[:, tile_win[tid], :],
                                start=True, stop=True,
                                skip_group_check=True)
                        # k+e | v+e into SBUF bf16
                        kvs = ebuf.tile([P, B, 2 * D], bf16, tag="kvs",
                                        name="kvs")
                        nc.vector.tensor_tensor(
                            out=kvs[:, :nb, :], in0=kvg[:, qs:qs + nb, :],
                            in1=ep[:, :nb, :], op=OP.add)
                        qgs = ebuf.tile([P, B, D], bf16, tag="qgs",
                                        name="qgs")
                        nc.scalar.copy(out=qgs[:, :nb, :],
                                       in_=qgp[:, :nb, :])
                        qk = ebuf.tile([P, B, D], bf16, tag="qk", name="qk")
                        nc.vector.tensor_tensor(
                            out=qk[:, :nb, :].rearrange(
                                "p b (h c) -> p b h c", h=H),
                            in0=qgs[:, :nb, :].rearrange(
                                "p b (h c) -> p b h c", h=H),
                            in1=kvs[:, :nb, :D].rearrange(
                                "p b (h c) -> p b h c", h=H),
                            op=OP.mult)
                        al = ebuf.tile([P, B, H], fp32, tag="al", name="al")
                        nc.vector.tensor_reduce(
                            out=al[:, :nb, :],
                            in_=qk[:, :nb, :].rearrange(
                                "p b (h c) -> p b h c", h=H),
                            axis=AX.X, op=OP.add)
                        pk = ebuf.tile([P, B, D + 8], bf16, tag="pk",
                                       name="pk")
                        nc.scalar.activation(
                            out=pk[:, :nb, D:], in_=al[:, :nb, :],
                            func=AF.Exp, scale=1.0 / math.sqrt(C))
                        nc.vector.tensor_tensor(
                            out=pk[:, :nb, :D].rearrange(
                                "p b (h c) -> p b h c", h=H),
                            in0=kvs[:, :nb, D:].rearrange(
                                "p b (h c) -> p b h c", h=H),
                            in1=_bcast4(pk[:, :nb, D:], C),
                            op=OP.mult)
                        for u in range(nb):
                            tid = t0 + qs + u
                            w = tile_win[tid]
                            nc.tensor.matmul(
                                out=acc_t[:, w % 2, :],
                                lhsT=oh[:, qs + u, :],
                                rhs=pk[:, u, :],
                                start=win_first[tid], stop=win_last[tid],
                                skip_group_check=True)
                            if not win_last[tid]:
                                continue
                            ac = acc_t[:, w % 2, :]
                            dn = nsm.tile([P, H], fp32, tag="dn", name="dn")
                            nc.vector.tensor_scalar_add(dn[:], ac[:, D:],
                                                        1e-16)
                            rd = nsm.tile([P, H], fp32, tag="rd", name="rd")
                            nc.vector.reciprocal(out=rd[:], in_=dn[:])
                            mg = ntmp.tile([P, D], fp32, tag="mg", name="mg")
                            nc.vector.tensor_tensor(
                                out=mg[:].rearrange("p (h c) -> p h c", h=H),
                                in0=ac[:, :D].rearrange(
                                    "p (h c) -> p h c", h=H),
                                in1=_bcast3(rd[:], C), op=OP.mult)
                            nc.vector.tensor_tensor(
                                out=hc_t[:, w, :], in0=mg[:],
                                in1=skip_t[:, w, :], op=OP.add)
                            # group boundary: queue FFN (+ next node
                            # phase or head) for the completed windows
                            if w in grp_end_w:
                                ts = list(range(w - GW + 1, w + 1))
                                pending.append(
                                    lambda l=l, ts=ts: ffn_a_group(l, ts))
                                pending.append(
                                    lambda l=l, ts=ts: ffn_b_group(l, ts))
                                if l < L - 1:
                                    pending.append(
                                        lambda l=l, ts=ts:
                                        node_group(l + 1, ts))
                                else:
                                    pending.append(
                                        lambda ts=ts: head_group(ts))
                for f in pending:
                    f()

    nc.compile()
    return nc


def prep_inputs(x, edge_index, edge_attr,
                lin0_w, lin0_b,
                q_w, q_b, k_w, k_b, v_w, v_b, e_w, skip_w, skip_b,
                ln1_g, ln1_b, lins_w, lins_b, ln2_g, ln2_b,
                lins2_w, lins2_b, lnl_g, lnl_b, linl_w, linl_b):
    """Host-side sharding/sorting/folding."""
    x = np.asarray(x, np.float32)
    ei = np.asarray(edge_index, np.int64)
    ea = np.asarray(edge_attr, np.float32)
    src, dst = ei[0], ei[1]
    core = dst // NL
    slot = dst - core * NL

    def fold(W, bias, g, b):
        W = np.asarray(W, np.float64)
        Wf = W * np.asarray(g, np.float64)[None, :]
        cf = np.asarray(bias, np.float64) + W @ np.asarray(b, np.float64)
        return Wf.astype(np.float32), cf.astype(np.float32)

    w4T = np.zeros((L, D, 4 * D), np.float32)
    w1T = np.zeros((L, D, D), np.float32)
    w2T = np.zeros((L, D, D), np.float32)
    ewdT = np.zeros((L, ED, 2 * D), np.float32)
    zero_bias = True
    for l in range(L):
        for j, (W, bias) in enumerate([(q_w[l], q_b[l]), (k_w[l], k_b[l]),
                                       (v_w[l], v_b[l]),
                                       (skip_w[l], skip_b[l])]):
            Wf, cf = fold(W, bias, ln1_g[l], ln1_b[l])
            w4T[l, :, j * D:(j + 1) * D] = Wf.T
            zero_bias &= bool(np.abs(cf).max() == 0)
        w1T[l] = np.asarray(lins_w[l]).T
        zero_bias &= bool(np.abs(np.asarray(lins_b[l])).max() == 0)
        Wf, cf = fold(lins2_w[l], lins2_b[l], ln2_g[l], ln2_b[l])
        w2T[l] = Wf.T
        zero_bias &= bool(np.abs(cf).max() == 0)
        ewT = np.asarray(e_w[l]).T.astype(np.float32)   # [ED, D]
        ewdT[l, :, :D] = ewT
        ewdT[l, :, D:] = ewT
    Wl, cl = fold(linl_w, linl_b, lnl_g, lnl_b)
    wlT = np.zeros((D, 4), np.float32)
    wlT[:, :3] = Wl.T
    zero_bias &= bool(np.abs(cl).max() == 0)
    zero_bias &= bool(np.abs(np.asarray(lin0_b)).max() == 0)

    win = slot // 128
    counts = np.zeros((NCORES, NT), np.int64)
    np.add.at(counts, (core, win), 1)
    tiles_per_window = [max(1, int(math.ceil(counts[:, w].max() / 128)))
                        for w in range(NT)]
    tot_tiles = sum(tiles_per_window)
    tot_e = tot_tiles * 128

    in_maps = []
    order_all = np.lexsort((win, core))
    off = np.searchsorted(core[order_all], np.arange(NCORES + 1))
    kvrow_of = (src // NL) * NLP + (src % NL)
    rowidx = np.tile(np.arange(P, dtype=np.float32), (P, 1)).copy()

    for c in range(NCORES):
        oc = order_all[off[c]:off[c + 1]]
        wc = win[oc]
        woff = np.searchsorted(wc, np.arange(NT + 1))
        src_rows = np.zeros(tot_e, np.int16)
        dst_rel = np.full(tot_e, -1.0, np.float32)   # slot within window
        ea_t = np.zeros((ED, tot_e), np.float32)
        base = 0
        for w in range(NT):
            ew_idx = oc[woff[w]:woff[w + 1]]
            k = len(ew_idx)
            sl = slice(base, base + k)
            src_rows[sl] = kvrow_of[ew_idx].astype(np.int16)
            dst_rel[sl] = (slot[ew_idx] - w * 128).astype(np.float32)
            ea_t[:, sl] = ea[ew_idx].T
            base += tiles_per_window[w] * 128
        assert base == tot_e

        ohT = np.zeros((P, tot_e), np.float32)
        real = dst_rel >= 0
        ohT[dst_rel[real].astype(np.int64), np.nonzero(real)[0]] = 1.0
        dst_col = dst_rel.reshape(tot_tiles, P).T.copy()  # [P, tot_tiles]

        def wrap(a):
            return np.tile(a.reshape(tot_e // 16, 16).T, (8, 1)).copy()

        xs = np.zeros((NLP, F), np.float32)
        xs[:NL] = x[c * NL:(c + 1) * NL]
        in_maps.append({
            "x_shard": xs,
            "idx_src": wrap(src_rows),
            "ohT": ohT,
            "dst_col": dst_col,
            "rowidx": rowidx,
            "ea_t": ea_t,
            "w4T": w4T, "w1T": w1T, "w2T": w2T, "ewdT": ewdT,
            "w0T": np.asarray(lin0_w).T.astype(np.float32),
            "ident": np.eye(P, dtype=np.float32),
            "wlT": wlT,
        })
    return in_maps, tiles_per_window, zero_bias


_CACHE = {}


def kernel(**inputs):
    import ml_dtypes
    in_maps, tiles_per_window, zero_bias = prep_inputs(**inputs)
    for m in in_maps:
        for k in ("ohT", "dst_col", "rowidx", "ea_t",
                  "w4T", "w1T", "w2T", "ewdT", "wlT"):
            m[k] = m[k].astype(ml_dtypes.bfloat16)

    key = tuple(tiles_per_window)
    if key not in _CACHE:
        _CACHE[key] = build(tiles_per_window, zero_bias)
    nc = _CACHE[key]

    global LAST_RESULT
    res = run_bass_kernel_spmd(
        nc, in_maps, core_ids=list(range(NCORES)),
        trace=bool(os.environ.get("K_TRACE")))
    LAST_RESULT = res
    out = np.zeros((N, 3), np.float32)
    for c in range(NCORES):
        out[c * NL:(c + 1) * NL] = res.results[c]["out"][:NL, :3]
    return out


# revision 23
# speedup vs baseline: 1.0568x; 1.0568x over previous
"""TransformerConv GNN (3 layers) on 8 Trainium2 NeuronCores.

Sharding: nodes split 3750/core (padded to 3840 = 30 tiles of 128).
Edges assigned to the core owning their dst node, grouped by 128-node
dst windows.

Per layer:
  node phase: ln1 (folded into weights host-side); fused q|k|v|skip
    projection as ONE bf16 matmul per tile ([D, 4D] moving operand);
    transposes via the DMA xbar (dma_start_transpose), not the PE;
    interleaved k|v table written to HBM (bf16).
  kv exchange: AllGather of the per-core kv shard (bf16).
  edge phase: dma_gather of kv[src] only. Per-edge q comes from a PE
    matmul with the transposed dst one-hot as stationary (dst is
    window-local). The scatter one-hot is generated on-chip (DVE
    is_equal vs an iota row). Edge-attr projection e goes PE->PSUM;
    k+e / v+e computed by one DVE add into SBUF bf16; alpha/exp/message
    on DVE+ACT; segment softmax accumulated per dst window via one-hot
    matmuls into PSUM.
  FFN phase + next layer's node phase are interleaved into the edge
    phase in groups of 10 dst windows, so they hide under the
    Q7-gather-bound edge phase; activations batched per group to keep
    ACT function-table reloads rare.
Output head node-local (also per group); host reassembles shards.
"""
import contextlib
import math
import os
import numpy as np

import concourse.bass as bass
import concourse.bacc as bacc
import concourse.tile as tile
from concourse import mybir, library_config
from concourse.bass_utils import run_bass_kernel_spmd

# problem dims
N, E, F, D, H, C, ED, L = 30000, 300000, 64, 128, 8, 16, 16, 3
NCORES = 8
NL = N // NCORES          # 3750 real nodes per core
NT = 30                   # node tiles per core
NLP = NT * 128            # 3840 padded nodes per core
KVROWS = NCORES * NLP     # kv table rows (global)
P = 128
G = 8                     # edge tiles per gather batch (dma_gather <=1024 idxs)
B = 4                     # edge tiles per DVE op group
GW = 10                   # dst windows per interleaved node/FFN group

fp32 = mybir.dt.float32
bf16 = mybir.dt.bfloat16
i16 = mybir.dt.int16

AF = mybir.ActivationFunctionType
OP = mybir.AluOpType
AX = mybir.AxisListType

LAST_RESULT = None


def _bcast3(ap, reps):
    """[P, k] AP -> [P, k, reps] with 0-stride last dim."""
    return bass.AP(tensor=ap.tensor, offset=ap.offset,
                   ap=[ap.ap[0], ap.ap[1], [0, reps]])


def _bcast4(ap, reps):
    """[P, b, k] AP -> [P, b, k, reps] with 0-stride last dim."""
    return bass.AP(tensor=ap.tensor, offset=ap.offset,
                   ap=[ap.ap[0], ap.ap[1], ap.ap[2], [0, reps]])


def _bcast_mid(ap, reps):
    """[P, k] AP -> [P, reps, k] with 0-stride middle dim."""
    return bass.AP(tensor=ap.tensor, offset=ap.offset,
                   ap=[ap.ap[0], [0, reps], ap.ap[1]])


def build(tiles_per_window, skip_bias):
    """Build the Bass program. tiles_per_window: NT ints, same per core."""
    assert skip_bias, "non-zero bias path not implemented"
    tot_tiles = sum(tiles_per_window)
    tot_e = tot_tiles * 128
    nbatch = math.ceil(tot_tiles / G)

    tile_win, win_first, win_last = [], [], []
    for w, tw in enumerate(tiles_per_window):
        for i in range(tw):
            tile_win.append(w)
            win_first.append(i == 0)
            win_last.append(i == tw - 1)

    nc = bacc.Bacc("TRN2", target_bir_lowering=False, debug=False,
                   num_devices=NCORES)

    # ---------------- DRAM tensors ----------------
    x_in = nc.dram_tensor("x_shard", [NLP, F], fp32, kind="ExternalInput").ap()
    idx_src_d = nc.dram_tensor("idx_src", [P, tot_e // 16], i16,
                               kind="ExternalInput").ap()
    ohT_d = nc.dram_tensor("ohT", [P, tot_e], bf16, kind="ExternalInput").ap()
    dcol_d = nc.dram_tensor("dst_col", [P, tot_tiles], bf16,
                            kind="ExternalInput").ap()
    rix_d = nc.dram_tensor("rowidx", [P, P], bf16, kind="ExternalInput").ap()
    ea_d = nc.dram_tensor("ea_t", [ED, tot_e], bf16, kind="ExternalInput").ap()
    w4_d = nc.dram_tensor("w4T", [L, D, 4 * D], bf16,
                          kind="ExternalInput").ap()
    w1_d = nc.dram_tensor("w1T", [L, D, D], bf16, kind="ExternalInput").ap()
    w2_d = nc.dram_tensor("w2T", [L, D, D], bf16, kind="ExternalInput").ap()
    ewd_d = nc.dram_tensor("ewdT", [L, ED, 2 * D], bf16,
                           kind="ExternalInput").ap()
    w0_d = nc.dram_tensor("w0T", [F, D], fp32, kind="ExternalInput").ap()
    id_d = nc.dram_tensor("ident", [P, P], fp32, kind="ExternalInput").ap()
    wl_d = nc.dram_tensor("wlT", [D, 4], bf16, kind="ExternalInput").ap()
    out_d = nc.dram_tensor("out", [NLP, 4], fp32, kind="ExternalOutput").ap()

    kv_bounce = nc.dram_tensor("kv_bounce", [NLP, 2 * D], bf16).ap()
    kv_full = [nc.dram_tensor(f"kv_full{i}", [KVROWS, 2 * D], bf16,
                              addr_space="Shared").ap() for i in range(2)]
    CH = NT // GW             # kv-exchange chunks per layer
    CROWS = NLP // CH         # kv_bounce rows per chunk

    eps = 1e-5

    # group boundaries: window index at which each group completes
    grp_end_w = [GW - 1 + GW * i for i in range(NT // GW)]  # [9, 19, 29]

    with tile.TileContext(nc) as tc:
        nc.gpsimd.load_library(library_config.mlp)
        with contextlib.ExitStack() as ctx:
            const = ctx.enter_context(tc.tile_pool(name="const", bufs=1))
            nodes = ctx.enter_context(tc.tile_pool(name="nodes", bufs=1))
            ntmp = ctx.enter_context(tc.tile_pool(name="ntmp", bufs=3))
            nsm = ctx.enter_context(tc.tile_pool(name="nsm", bufs=4))
            gtmp = ctx.enter_context(tc.tile_pool(name="gtmp", bufs=2))
            gbuf = ctx.enter_context(tc.tile_pool(name="gbuf", bufs=3))
            ebuf = ctx.enter_context(tc.tile_pool(name="ebuf", bufs=3))
            # PSUM pools (live across the whole kernel)
            pps = ctx.enter_context(
                tc.tile_pool(name="pps", bufs=1, space="PSUM"))

            # constants
            id32 = const.tile([P, P], fp32, tag="id32")
            nc.sync.dma_start(out=id32[:], in_=id_d[:, :])
            id16 = const.tile([P, P], bf16, tag="id16")
            nc.vector.tensor_copy(out=id16[:], in_=id32[:])
            eps_t = const.tile([P, 1], fp32, tag="eps")
            nc.vector.memset(eps_t[:], eps)
            rix = const.tile([P, P], bf16, tag="rix")
            nc.sync.dma_start(out=rix[:], in_=rix_d[:, :])
            dcol = const.tile([P, tot_tiles], bf16, tag="dcol")
            nc.sync.dma_start(out=dcol[:], in_=dcol_d[:, :])
            idx_src = const.tile([P, tot_e // 16], i16, tag="isrc")
            nc.sync.dma_start(out=idx_src[:], in_=idx_src_d[:, :])
            w0 = const.tile([F, D], fp32, tag="w0")
            nc.sync.dma_start(out=w0[:], in_=w0_d[:, :])
            wl = const.tile([D, 4], bf16, tag="wl")
            nc.sync.dma_start(out=wl[:], in_=wl_d[:, :])
            w4 = const.tile([D, L, 4 * D], bf16, tag="w4")
            w1 = const.tile([D, L, D], bf16, tag="w1")
            w2 = const.tile([D, L, D], bf16, tag="w2")
            ewd = const.tile([ED, L, 2 * D], bf16, tag="ewd")
            for l in range(L):
                nc.sync.dma_start(out=w4[:, l, :], in_=w4_d[l])
                nc.sync.dma_start(out=w1[:, l, :], in_=w1_d[l])
                nc.sync.dma_start(out=w2[:, l, :], in_=w2_d[l])
                nc.sync.dma_start(out=ewd[:, l, :], in_=ewd_d[l])

            acc_t = pps.tile([P, 2, D + 8], fp32, space="PSUM", tag="acc",
                             bufs=1, name="acc_t")
            h_t = nodes.tile([P, NT, D], fp32, tag="h")
            skip_t = nodes.tile([P, NT, D], fp32, tag="skip")
            idn_t = nodes.tile([P, NT, D], fp32, tag="idn")
            hc_t = nodes.tile([P, NT, D], bf16, tag="hc")
            q_sb = nodes.tile([P, NT, D], bf16, tag="qsb")

            def ag_chunk(lt, j):
                nc.gpsimd.collective_compute(
                    "AllGather", OP.bypass,
                    replica_groups=[list(range(NCORES))],
                    ins=[kv_bounce[j * CROWS:(j + 1) * CROWS, :].opt()],
                    outs=[kv_full[lt % 2][j * CROWS * NCORES:
                                          (j + 1) * CROWS * NCORES, :].opt()])

            def transpose16(src_ap, name):
                tp = pps.tile([P, P], bf16, space="PSUM", tag="tp",
                              bufs=1, name="tp_" + name)
                nc.tensor.transpose(out=tp[:], in_=src_ap, identity=id16[:])
                dst = ntmp.tile([P, P], bf16, tag="trT", name=name)
                nc.scalar.copy(out=dst[:], in_=tp[:])
                return dst

            def ln_group(src_t, ts, out_b):
                """Batched layernorm of src_t[:, t, :] for t in ts into
                out_b[:, i, :] (bf16)."""
                n = len(ts)
                sts = gtmp.tile([P, GW, 6], fp32, tag="sts", name="sts")
                mvs = gtmp.tile([P, GW, 2], fp32, tag="mvs", name="mvs")
                for i, t in enumerate(ts):
                    nc.vector.bn_stats(out=sts[:, i, :], in_=src_t[:, t, :])
                    nc.vector.bn_aggr(out=mvs[:, i, :], in_=sts[:, i, :])
                sd = gtmp.tile([P, GW], fp32, tag="sd", name="sd")
                nc.scalar.activation(out=sd[:, :n], in_=mvs[:, :n, 1:2],
                                     func=AF.Ln, bias=eps_t[:], scale=1.0)
                rs = gtmp.tile([P, GW], fp32, tag="rs", name="rs")
                nc.scalar.activation(out=rs[:, :n], in_=sd[:, :n],
                                     func=AF.Exp, scale=-0.5)
                for i, t in enumerate(ts):
                    nc.vector.scalar_tensor_tensor(
                        out=out_b[:, i, :], in0=src_t[:, t, :],
                        scalar=mvs[:, i, 0:1],
                        in1=rs[:, i:i + 1].to_broadcast([P, D]),
                        op0=OP.subtract, op1=OP.mult)

            def node_group(l, ts):
                """ln1 + fused qkvs projection for tiles ts of layer l."""
                hnb = gtmp.tile([P, GW, D], bf16, tag="hnb", name="hnb")
                ln_group(h_t, ts, hnb)
                for i, t in enumerate(ts):
                    hnT = transpose16(hnb[:, i, :], "hnT")
                    p4 = pps.tile([P, 4 * D], fp32, space="PSUM", tag="p4",
                                  bufs=1, name="p4")
                    nc.tensor.matmul(out=p4[:], lhsT=hnT[:], rhs=w4[:, l, :],
                                     start=True, stop=True)
                    nc.scalar.copy(out=q_sb[:, t, :], in_=p4[:, :D])
                    kvb = ntmp.tile([P, 2 * D], bf16, tag="kvb", name="kvb")
                    nc.scalar.copy(out=kvb[:], in_=p4[:, D:3 * D])
                    nc.sync.dma_start(out=kv_bounce[t * P:(t + 1) * P, :],
                                      in_=kvb[:])
                    nc.vector.tensor_copy(out=skip_t[:, t, :],
                                          in_=p4[:, 3 * D:])

            def ffn_a_group(l, ts):
                """idn = gelu(hc @ w1) + h for tiles ts."""
                for t in ts:
                    hcT = transpose16(hc_t[:, t, :], "hcT")
                    t1p = pps.tile([P, D], fp32, space="PSUM", tag="mm",
                                   bufs=2, name="t1p")
                    nc.tensor.matmul(out=t1p[:], lhsT=hcT[:], rhs=w1[:, l, :],
                                     start=True, stop=True)
                    t1g = ntmp.tile([P, D], fp32, tag="t1g", name="t1g")
                    nc.scalar.activation(out=t1g[:], in_=t1p[:], func=AF.Gelu)
                    nc.vector.tensor_tensor(out=idn_t[:, t, :], in0=t1g[:],
                                            in1=h_t[:, t, :], op=OP.add)

            def ffn_b_group(l, ts):
                """h = elu(ln2(idn) @ w2) + idn for tiles ts."""
                t2b = gtmp.tile([P, GW, D], bf16, tag="hnb", name="t2b")
                ln_group(idn_t, ts, t2b)
                for i, t in enumerate(ts):
                    t2T = transpose16(t2b[:, i, :], "t2T")
                    t3p = pps.tile([P, D], fp32, space="PSUM", tag="mm",
                                   bufs=2, name="t3p")
                    nc.tensor.matmul(out=t3p[:], lhsT=t2T[:], rhs=w2[:, l, :],
                                     start=True, stop=True)
                    mn = nsm.tile([P, D], fp32, tag="mn", name="mn2")
                    nc.vector.tensor_scalar_min(mn[:], t3p[:], 0.0)
                    em = nsm.tile([P, D], fp32, tag="em", name="em2")
                    nc.scalar.activation(out=em[:], in_=mn[:], func=AF.Exp)
                    mx = nsm.tile([P, D], fp32, tag="mx", name="mx2")
                    nc.vector.tensor_scalar_max(mx[:], t3p[:], 0.0)
                    t4 = nsm.tile([P, D], fp32, tag="t4", name="t4")
                    nc.vector.scalar_tensor_tensor(
                        out=t4[:], in0=em[:], scalar=-1.0, in1=mx[:],
                        op0=OP.add, op1=OP.add)
                    nc.vector.tensor_tensor(out=h_t[:, t, :], in0=t4[:],
                                            in1=idn_t[:, t, :], op=OP.add)

            def head_group(ts):
                """Final layernorm + output projection for tiles ts."""
                hlb = gtmp.tile([P, GW, D], bf16, tag="hnb", name="hlb")
                ln_group(h_t, ts, hlb)
                for i, t in enumerate(ts):
                    hlT = transpose16(hlb[:, i, :], "hlT")
                    opm = pps.tile([P, D], fp32, space="PSUM", tag="mm",
                                   bufs=2, name="opm")
                    op_ = opm[:, 0:4]
                    nc.tensor.matmul(out=op_, lhsT=hlT[:], rhs=wl[:],
                                     start=True, stop=True)
                    ot = ntmp.tile([P, 4], fp32, tag="ot", name="ot")
                    nc.scalar.copy(out=ot[:], in_=op_)
                    nc.sync.dma_start(out=out_d[t * P:(t + 1) * P, :],
                                      in_=ot[:])

            # ---------------- input projection ----------------
            for t in range(NT):
                xt = ntmp.tile([P, F], fp32, tag="xt", name="xt")
                nc.sync.dma_start(out=xt[:], in_=x_in[t * P:(t + 1) * P, :])
                tp = pps.tile([P, P], fp32, space="PSUM", tag="mm",
                              bufs=2, name="tp")
                nc.tensor.transpose(out=tp[:F, :], in_=xt[:], identity=id32[:])
                xT = ntmp.tile([F, P], fp32, tag="xT", name="xT")
                nc.vector.tensor_copy(out=xT[:], in_=tp[:F, :])
                h0 = pps.tile([P, D], fp32, space="PSUM", tag="mm",
                              bufs=2, name="h0")
                nc.tensor.matmul(out=h0[:], lhsT=xT[:], rhs=w0[:],
                                 start=True, stop=True)
                mn = nsm.tile([P, D], fp32, tag="mn", name="mn")
                nc.vector.tensor_scalar_min(mn[:], h0[:], 0.0)
                em = nsm.tile([P, D], fp32, tag="em", name="em")
                nc.scalar.activation(out=em[:], in_=mn[:], func=AF.Exp)
                mx = nsm.tile([P, D], fp32, tag="mx", name="mx")
                nc.vector.tensor_scalar_max(mx[:], h0[:], 0.0)
                nc.vector.scalar_tensor_tensor(
                    out=h_t[:, t, :], in0=em[:], scalar=-1.0, in1=mx[:],
                    op0=OP.add, op1=OP.add)

            # layer 0 node phase (later layers interleave into edge phase)
            for g0 in range(0, NT, GW):
                node_group(0, list(range(g0, g0 + GW)))
                ag_chunk(0, g0 // GW)

            # ---------------- layers ----------------
            for l in range(L):
                # ---- edge phase with interleaved FFN / next node phase ----
                pending = []
                for g in range(nbatch):
                    if pending:
                        pending.pop(0)()
                    t0 = g * G
                    gb = min(G, tot_tiles - t0)
                    ne = gb * 128
                    kvg = gbuf.tile([P, G, 2 * D], bf16, tag="kvg",
                                    name="kvg")
                    nc.gpsimd.dma_gather(
                        kvg[:, :gb, :], kv_full[l % 2][:],
                        idx_src[:, t0 * 8:t0 * 8 + ne // 16],
                        ne, ne, 2 * D)
                    eat = gbuf.tile([ED, G * 128], bf16, tag="eat",
                                    name="eat")
                    nc.sync.dma_start(
                        out=eat[:, :ne],
                        in_=ea_d[:, t0 * 128:t0 * 128 + ne])
                    oh = gbuf.tile([P, G, P], bf16, tag="oh", name="oh")
                    nc.vector.tensor_tensor(
                        out=oh[:, :gb, :],
                        in0=_bcast_mid(rix[:], gb),
                        in1=_bcast3(dcol[:, t0:t0 + gb], P),
                        op=OP.is_equal)
                    ohTs = gbuf.tile([P, G, P], bf16, tag="ohTs",
                                     name="ohTs")
                    nc.sync.dma_start(
                        out=ohTs[:, :gb, :],
                        in_=ohT_d[:, t0 * 128:t0 * 128 + ne])

                    for bb in range(math.ceil(gb / B)):
                        nb = min(B, gb - bb * B)
                        qs = bb * B
                        ep = pps.tile([P, B, 2 * D], fp32, space="PSUM",
                                      tag="ep", bufs=1, name="ep")
                        qgp = pps.tile([P, B, D], fp32, space="PSUM",
                                       tag="qgp", bufs=1, name="qgp")
                        for u in range(nb):
                            te = qs + u
                            tid = t0 + te
                            nc.tensor.matmul(
                                out=ep[:, u, :],
                                lhsT=eat[:, te * 128:(te + 1) * 128],
                                rhs=ewd[:, l, :], start=True, stop=True,
                                skip_group_check=True)
                            nc.tensor.matmul(
                                out=qgp[:, u, :],
                                lhsT=ohTs[:, te, :],
                                rhs=q_sb[:, tile_win[tid], :],
                                start=True, stop=True,
                                skip_group_check=True)
                        # k+e | v+e into SBUF bf16
                        kvs = ebuf.tile([P, B, 2 * D], bf16, tag="kvs",
                                        name="kvs")
                        nc.vector.tensor_tensor(
                            out=kvs[:, :nb, :], in0=kvg[:, qs:qs + nb, :],
                            in1=ep[:, :nb, :], op=OP.add)
                        qgs = ebuf.tile([P, B, D], bf16, tag="qgs",
                                        name="qgs")
                        nc.scalar.copy(out=qgs[:, :nb, :],
                                       in_=qgp[:, :nb, :])
                        qk = ebuf.tile([P, B, D], bf16, tag="qk", name="qk")
                        nc.vector.tensor_tensor(
                            out=qk[:, :nb, :].rearrange(
                                "p b (h c) -> p b h c", h=H),
                            in0=qgs[:, :nb, :].rearrange(
                                "p b (h c) -> p b h c", h=H),
                            in1=kvs[:, :nb, :D].rearrange(
                                "p b (h c) -> p b h c", h=H),
                            op=OP.mult)
                        al = ebuf.tile([P, B, H], fp32, tag="al", name="al")
                        nc.vector.tensor_reduce(
                            out=al[:, :nb, :],
                            in_=qk[:, :nb, :].rearrange(
                                "p b (h c) -> p b h c", h=H),
                            axis=AX.X, op=OP.add)
                        pk = ebuf.tile([P, B, D + 8], bf16, tag="pk",
                                       name="pk")
                        nc.scalar.activation(
                            out=pk[:, :nb, D:], in_=al[:, :nb, :],
                            func=AF.Exp, scale=1.0 / math.sqrt(C))
                        nc.vector.tensor_tensor(
                            out=pk[:, :nb, :D].rearrange(
                                "p b (h c) -> p b h c", h=H),
                            in0=kvs[:, :nb, D:].rearrange(
                                "p b (h c) -> p b h c", h=H),
                            in1=_bcast4(pk[:, :nb, D:], C),
                            op=OP.mult)
                        for u in range(nb):
                            tid = t0 + qs + u
                            w = tile_win[tid]
                            nc.tensor.matmul(
                                out=acc_t[:, w % 2, :],
                                lhsT=oh[:, qs + u, :],
                                rhs=pk[:, u, :],
                                start=win_first[tid], stop=win_last[tid],
                                skip_group_check=True)
                            if not win_last[tid]:
                                continue
                            ac = acc_t[:, w % 2, :]
                            dn = nsm.tile([P, H], fp32, tag="dn", name="dn")
                            nc.vector.tensor_scalar_add(dn[:], ac[:, D:],
                                                        1e-16)
                            rd = nsm.tile([P, H], fp32, tag="rd", name="rd")
                            nc.vector.reciprocal(out=rd[:], in_=dn[:])
                            mg = ntmp.tile([P, D], fp32, tag="mg", name="mg")
                            nc.vector.tensor_tensor(
                                out=mg[:].rearrange("p (h c) -> p h c", h=H),
                                in0=ac[:, :D].rearrange(
                                    "p (h c) -> p h c", h=H),
                                in1=_bcast3(rd[:], C), op=OP.mult)
                            nc.vector.tensor_tensor(
                                out=hc_t[:, w, :], in0=mg[:],
                                in1=skip_t[:, w, :], op=OP.add)
                            # group boundary: queue FFN (+ next node
                            # phase or head) for the completed windows
                            if w in grp_end_w:
                                ts = list(range(w - GW + 1, w + 1))
                                pending.append(
                                    lambda l=l, ts=ts: ffn_a_group(l, ts))
                                pending.append(
                                    lambda l=l, ts=ts: ffn_b_group(l, ts))
                                if l < L - 1:
                                    def nb(l=l, ts=ts):
                                        node_group(l + 1, ts)
                                        ag_chunk(l + 1, ts[0] // GW)
                                    pending.append(nb)
                                else:
                                    pending.append(
                                        lambda ts=ts: head_group(ts))
                for f in pending:
                    f()

    nc.compile()
    return nc


def prep_inputs(x, edge_index, edge_attr,
                lin0_w, lin0_b,
                q_w, q_b, k_w, k_b, v_w, v_b, e_w, skip_w, skip_b,
                ln1_g, ln1_b, lins_w, lins_b, ln2_g, ln2_b,
                lins2_w, lins2_b, lnl_g, lnl_b, linl_w, linl_b):
    """Host-side sharding/sorting/folding."""
    x = np.asarray(x, np.float32)
    ei = np.asarray(edge_index, np.int64)
    ea = np.asarray(edge_attr, np.float32)
    src, dst = ei[0], ei[1]
    core = dst // NL
    slot = dst - core * NL

    def fold(W, bias, g, b):
        W = np.asarray(W, np.float64)
        Wf = W * np.asarray(g, np.float64)[None, :]
        cf = np.asarray(bias, np.float64) + W @ np.asarray(b, np.float64)
        return Wf.astype(np.float32), cf.astype(np.float32)

    w4T = np.zeros((L, D, 4 * D), np.float32)
    w1T = np.zeros((L, D, D), np.float32)
    w2T = np.zeros((L, D, D), np.float32)
    ewdT = np.zeros((L, ED, 2 * D), np.float32)
    zero_bias = True
    for l in range(L):
        for j, (W, bias) in enumerate([(q_w[l], q_b[l]), (k_w[l], k_b[l]),
                                       (v_w[l], v_b[l]),
                                       (skip_w[l], skip_b[l])]):
            Wf, cf = fold(W, bias, ln1_g[l], ln1_b[l])
            w4T[l, :, j * D:(j + 1) * D] = Wf.T
            zero_bias &= bool(np.abs(cf).max() == 0)
        w1T[l] = np.asarray(lins_w[l]).T
        zero_bias &= bool(np.abs(np.asarray(lins_b[l])).max() == 0)
        Wf, cf = fold(lins2_w[l], lins2_b[l], ln2_g[l], ln2_b[l])
        w2T[l] = Wf.T
        zero_bias &= bool(np.abs(cf).max() == 0)
        ewT = np.asarray(e_w[l]).T.astype(np.float32)   # [ED, D]
        ewdT[l, :, :D] = ewT
        ewdT[l, :, D:] = ewT
    Wl, cl = fold(linl_w, linl_b, lnl_g, lnl_b)
    wlT = np.zeros((D, 4), np.float32)
    wlT[:, :3] = Wl.T
    zero_bias &= bool(np.abs(cl).max() == 0)
    zero_bias &= bool(np.abs(np.asarray(lin0_b)).max() == 0)

    win = slot // 128
    counts = np.zeros((NCORES, NT), np.int64)
    np.add.at(counts, (core, win), 1)
    tiles_per_window = [max(1, int(math.ceil(counts[:, w].max() / 128)))
                        for w in range(NT)]
    tot_tiles = sum(tiles_per_window)
    tot_e = tot_tiles * 128

    in_maps = []
    order_all = np.lexsort((win, core))
    off = np.searchsorted(core[order_all], np.arange(NCORES + 1))
    crows = NLP // 3          # chunk-major kv_full layout (must match build)
    s_slot = src % NL
    kvrow_of = ((s_slot // crows) * NCORES * crows
                + (src // NL) * crows + s_slot % crows)
    rowidx = np.tile(np.arange(P, dtype=np.float32), (P, 1)).copy()

    for c in range(NCORES):
        oc = order_all[off[c]:off[c + 1]]
        wc = win[oc]
        woff = np.searchsorted(wc, np.arange(NT + 1))
        src_rows = np.zeros(tot_e, np.int16)
        dst_rel = np.full(tot_e, -1.0, np.float32)   # slot within window
        ea_t = np.zeros((ED, tot_e), np.float32)
        base = 0
        for w in range(NT):
            ew_idx = oc[woff[w]:woff[w + 1]]
            k = len(ew_idx)
            sl = slice(base, base + k)
            src_rows[sl] = kvrow_of[ew_idx].astype(np.int16)
            dst_rel[sl] = (slot[ew_idx] - w * 128).astype(np.float32)
            ea_t[:, sl] = ea[ew_idx].T
            base += tiles_per_window[w] * 128
        assert base == tot_e

        ohT = np.zeros((P, tot_e), np.float32)
        real = dst_rel >= 0
        ohT[dst_rel[real].astype(np.int64), np.nonzero(real)[0]] = 1.0
        dst_col = dst_rel.reshape(tot_tiles, P).T.copy()  # [P, tot_tiles]

        def wrap(a):
            return np.tile(a.reshape(tot_e // 16, 16).T, (8, 1)).copy()

        xs = np.zeros((NLP, F), np.float32)
        xs[:NL] = x[c * NL:(c + 1) * NL]
        in_maps.append({
            "x_shard": xs,
            "idx_src": wrap(src_rows),
            "ohT": ohT,
            "dst_col": dst_col,
            "rowidx": rowidx,
            "ea_t": ea_t,
            "w4T": w4T, "w1T": w1T, "w2T": w2T, "ewdT": ewdT,
            "w0T": np.asarray(lin0_w).T.astype(np.float32),
            "ident": np.eye(P, dtype=np.float32),
            "wlT": wlT,
        })
    return in_maps, tiles_per_window, zero_bias


_CACHE = {}


def kernel(**inputs):
    import ml_dtypes
    in_maps, tiles_per_window, zero_bias = prep_inputs(**inputs)
    for m in in_maps:
        for k in ("ohT", "dst_col", "rowidx", "ea_t",
                  "w4T", "w1T", "w2T", "ewdT", "wlT"):
            m[k] = m[k].astype(ml_dtypes.bfloat16)

    key = tuple(tiles_per_window)
    if key not in _CACHE:
        _CACHE[key] = build(tiles_per_window, zero_bias)
    nc = _CACHE[key]

    global LAST_RESULT
    res = run_bass_kernel_spmd(
        nc, in_maps, core_ids=list(range(NCORES)),
        trace=bool(os.environ.get("K_TRACE")))
    LAST_RESULT = res
    out = np.zeros((N, 3), np.float32)
    for c in range(NCORES):
        out[c * NL:(c + 1) * NL] = res.results[c]["out"][:NL, :3]
    return out
